# revision 27
# baseline (speedup 1.0000x reference)
"""Mamba block + FFN on 8 Trainium2 NeuronCores (Bass/Tile, SPMD).

Self-contained: hardcoded shapes for nn_Block_472446402621.
  x [2, 4096, 1024] fp32 -> out [2, 4096, 1024] fp32

Three SPMD launches with host-side (free) reshuffling between:
  A: token-parallel  — LN1 + in_proj + causal conv + silu + x_proj + dt_proj
  B: channel-parallel — selective scan (tensor_tensor_scan) + gating
  C: token-parallel  — out_proj + residual + LN2 + FFN + residual
"""
import numpy as np
import ml_dtypes
from contextlib import ExitStack

import concourse.bass as bass
import concourse.tile as tile
from concourse import mybir
from concourse.bass_utils import run_bass_kernel_spmd

F32 = mybir.dt.float32
F32R = mybir.dt.float32r
BF16 = mybir.dt.bfloat16
AF = mybir.ActivationFunctionType
MULT = mybir.AluOpType.mult
ADD = mybir.AluOpType.add
SUB = mybir.AluOpType.subtract
AXX = mybir.AxisListType.X

B, T, D = 2, 4096, 1024
E, N, RK, H, KC = 1024, 16, 64, 4096, 8
BT = B * T
NC = 8
TOK = BT // NC          # tokens per core (A, C)
HALO = 8                # conv halo columns (first 8 of xs)
TOKH = TOK + HALO
ES = E // NC            # channels per core (B)
TC = 512                # scan chunk length
NCH = BT // TC
KT = D // 128           # k tiles over d_model
EPS = 1e-5

_cache = {}


# ---------------------------------------------------------------------------
# sync-wait legalization: walrus instruction encodings hold ~1 sem wait;
# hoist excess monotone waits into standalone EventSemaphore instructions.
def _fix_sync_waits(nc):
    cnt = 0
    for f in nc.m.functions:
        for bb in f.blocks:
            insts = bb.instructions
            out = []
            changed = False
            for inst in insts:
                si = inst.sync_info
                waits = list(si.on_wait) if si is not None else []
                if len(waits) > 1:
                    hoist, keep = [], []
                    for w in waits:
                        if w.wait_mode in ("sem-ge-imm", "sem-ge-reg"):
                            hoist.append(w)
                        else:
                            keep.append(w)
                    room = 1 - len(keep)
                    assert room >= 0, f"{inst.name}: non-monotone waits > 1"
                    keep += hoist[:room]
                    hoist = hoist[room:]
                    for w in hoist:
                        cnt += 1
                        ni = mybir.InstEventSemaphore(
                            name=f"W-fix-{cnt}", ins=[], outs=[],
                            sync_info=mybir.SyncInfo(on_wait=[w], on_update=[]))
                        ni.engine = inst.engine
                        out.append(ni)
                    inst.sync_info = mybir.SyncInfo(
                        on_wait=keep, on_update=list(si.on_update))
                    changed = True
                out.append(inst)
            if changed:
                bb.instructions = out
    return cnt


def _new_nc():
    return bass.Bass("TRN2", target_bir_lowering=False, debug=False,
                     num_devices=NC)


# ---------------------------------------------------------------------------
def build_launch_a():
    nc = _new_nc()
    xs = nc.dram_tensor("xs", [D, TOKH], F32, kind="ExternalInput").ap()
    wxz = nc.dram_tensor("wxz", [D, 2 * E], BF16, kind="ExternalInput").ap()
    wxp = nc.dram_tensor("wxp", [E, 96], BF16, kind="ExternalInput").ap()
    wdt = nc.dram_tensor("wdt", [RK, E], BF16, kind="ExternalInput").ap()
    cw = nc.dram_tensor("cw", [E, KC], F32, kind="ExternalInput").ap()
    cbias = nc.dram_tensor("cbias", [E, 1], F32, kind="ExternalInput").ap()
    dtb = nc.dram_tensor("dtb", [E, 1], F32, kind="ExternalInput").ap()
    bxz = nc.dram_tensor("bxz", [2 * E, 1], F32, kind="ExternalInput").ap()

    delta_s = nc.dram_tensor("delta_s", [E, TOK], F32, kind="ExternalOutput").ap()
    xc_s = nc.dram_tensor("xc_s", [E, TOK], BF16, kind="ExternalOutput").ap()
    zs_s = nc.dram_tensor("zs_s", [E, TOK], BF16, kind="ExternalOutput").ap()
    bc_s = nc.dram_tensor("bc_s", [N, TOK], BF16, kind="ExternalOutput").ap()
    cc_s = nc.dram_tensor("cc_s", [N, TOK], BF16, kind="ExternalOutput").ap()

    with tile.TileContext(nc) as tc, ExitStack() as ctx:
        pool = ctx.enter_context(tc.tile_pool(name="p", bufs=1))
        pev = ctx.enter_context(tc.tile_pool(name="ev", bufs=2))
        pcv = ctx.enter_context(tc.tile_pool(name="cv", bufs=1))
        pw = ctx.enter_context(tc.tile_pool(name="pw", bufs=3))
        psum = ctx.enter_context(tc.tile_pool(name="ps", bufs=1, space="PSUM"))
        psb = psum

        # ---- load x (8 k-tiles) ----
        xk = pool.tile([128, KT * TOKH], F32, tag="xk")
        nc.sync.dma_start(
            xk[:].rearrange("p (k t) -> p k t", k=KT),
            xs[:].rearrange("(k p) t -> p k t", p=128))
        # weights (wxz streamed per output e-tile later)
        wxp_sb = pool.tile([128, KT * 96], BF16, tag="wxp")
        nc.sync.dma_start(
            wxp_sb[:].rearrange("p (k e) -> p k e", k=KT),
            wxp[:].rearrange("(k p) e -> p k e", p=128))
        wdt_sb = pool.tile([RK, E], BF16, tag="wdt")
        nc.sync.dma_start(wdt_sb[:], wdt[:])
        cw_sb = pool.tile([128, KT * KC], F32, tag="cw")
        nc.sync.dma_start(
            cw_sb[:].rearrange("p (k c) -> p k c", k=KT),
            cw[:].rearrange("(k p) c -> p k c", p=128))
        cb_sb = pool.tile([128, KT], F32, tag="cb")
        nc.sync.dma_start(cb_sb[:].unsqueeze(2),
                          cbias[:].rearrange("(k p) o -> p k o", p=128))
        dtb_sb = pool.tile([128, KT], F32, tag="dtb")
        nc.sync.dma_start(dtb_sb[:].unsqueeze(2),
                          dtb[:].rearrange("(k p) o -> p k o", p=128))
        bxz_sb = pool.tile([128, 2 * KT], F32, tag="bxz")
        nc.sync.dma_start(bxz_sb[:].unsqueeze(2),
                          bxz[:].rearrange("(k p) o -> p k o", p=128))

        ones_k = pool.tile([128, 1], F32, tag="ones_k")
        nc.vector.memset(ones_k[:], 1.0)
        ones_r = pool.tile([1, 128], F32, tag="ones_r")
        nc.vector.memset(ones_r[:], 1.0)

        xk3 = xk[:].rearrange("p (k t) -> p k t", k=KT)

        # ---- LN1 stats (square streamed per k-tile) ----
        ps_x = psum.tile([1, 1536], F32, tag="pstat0")
        ps_q = psum.tile([1, 1536], F32, tag="pstat1")
        pieces = [(0, 512), (512, 512), (1024, HALO)]
        for kk in range(KT):
            sqt = pev.tile([128, TOKH], F32, tag="scratch")
            nc.scalar.activation(sqt[:], xk3[:, kk], AF.Square)
            for (o, w_) in pieces:
                nc.tensor.matmul(ps_x[:, o:o + w_], ones_k[:],
                                 xk3[:, kk, o:o + w_],
                                 start=(kk == 0), stop=(kk == KT - 1))
            for (o, w_) in pieces:
                nc.tensor.matmul(ps_q[:, o:o + w_], ones_k[:],
                                 sqt[:, o:o + w_],
                                 start=(kk == 0), stop=(kk == KT - 1))

        mu = pool.tile([1, TOKH], F32, tag="mu")
        nc.scalar.mul(mu[:], ps_x[:, 0:TOKH], 1.0 / D)
        ex2 = pool.tile([1, TOKH], F32, tag="ex2")
        nc.scalar.mul(ex2[:], ps_q[:, 0:TOKH], 1.0 / D)
        var = pool.tile([1, TOKH], F32, tag="var")
        nc.vector.tensor_tensor(var[:], mu[:], mu[:], op=MULT)
        nc.vector.tensor_tensor(var[:], ex2[:], var[:], op=SUB)
        epst = pool.tile([1, 1], F32, tag="epst")
        nc.vector.memset(epst[:], EPS)
        std = pool.tile([1, TOKH], F32, tag="std")
        nc.scalar.activation(std[:], var[:], AF.Sqrt, bias=epst[:])
        rstd = pool.tile([1, TOKH], F32, tag="rstd")
        nc.vector.reciprocal(rstd[:], std[:])
        ms = pool.tile([1, TOKH], F32, tag="ms")
        nc.vector.tensor_tensor(ms[:], mu[:], rstd[:], op=MULT)

        ps_rs = psum.tile([128, 1536], F32, tag="pstat0")
        ps_ms = psum.tile([128, 1536], F32, tag="pstat1")
        for (o, w_) in pieces:
            nc.tensor.matmul(ps_rs[:, o:o + w_], ones_r[:], rstd[:, o:o + w_],
                             start=True, stop=True)
            nc.tensor.matmul(ps_ms[:, o:o + w_], ones_r[:], ms[:, o:o + w_],
                             start=True, stop=True)

        # ---- xhat = x*rstd - mu*rstd   (bf16) ----
        xhat = pool.tile([128, KT * TOKH], BF16, tag="xhat")
        xhat3 = xhat[:].rearrange("p (k t) -> p k t", k=KT)
        tmp = pev.tile([128, TOKH], F32, tag="lntmp")
        for kk in range(KT):
            tmp = pev.tile([128, TOKH], F32, tag="lntmp")
            nc.vector.tensor_tensor(tmp[:], xk3[:, kk], ps_rs[:, 0:TOKH], op=MULT)
            nc.vector.tensor_tensor(xhat3[:, kk], tmp[:], ps_ms[:, 0:TOKH], op=SUB)

        # ---- in_proj: xin (e-tiles 0..7) and z (8..15), wxz streamed ----
        xin = pool.tile([128, KT * TOKH], BF16, tag="xin")
        xin3 = xin[:].rearrange("p (k t) -> p k t", k=KT)
        for et in range(KT):
            wm = pw.tile([128, KT * 128], BF16, tag="wmA")
            nc.sync.dma_start(
                wm[:].rearrange("p (k e) -> p k e", k=KT),
                wxz[:, et * 128:(et + 1) * 128].rearrange(
                    "(k p) e -> p k e", p=128))
            wm3 = wm[:].rearrange("p (k e) -> p k e", k=KT)
            ps = psb.tile([128, 1536], F32, tag=f"pstat{et % 2}")
            for (o, w_) in pieces:
                for kk in range(KT):
                    nc.tensor.matmul(
                        ps[:, o:o + w_], wm3[:, kk],
                        xhat3[:, kk, o:o + w_],
                        start=(kk == 0), stop=(kk == KT - 1))
            nc.scalar.activation(xin3[:, et], ps[:, 0:TOKH], AF.Identity,
                                 bias=bxz_sb[:, et:et + 1])
        for et in range(KT):
            wm = pw.tile([128, KT * 128], BF16, tag="wmA")
            nc.sync.dma_start(
                wm[:].rearrange("p (k e) -> p k e", k=KT),
                wxz[:, E + et * 128:E + (et + 1) * 128].rearrange(
                    "(k p) e -> p k e", p=128))
            wm3 = wm[:].rearrange("p (k e) -> p k e", k=KT)
            ps = psb.tile([128, 1536], F32, tag=f"pstat{et % 2}")
            for (o, w_) in pieces:
                for kk in range(KT):
                    nc.tensor.matmul(
                        ps[:, o:o + w_], wm3[:, kk],
                        xhat3[:, kk, o:o + w_],
                        start=(kk == 0), stop=(kk == KT - 1))
            zt = pev.tile([128, TOK], BF16, tag="zbf")
            nc.scalar.activation(zt[:], ps[:, HALO:TOKH], AF.Silu,
                                 bias=bxz_sb[:, KT + et:KT + et + 1])
            nc.scalar.dma_start(zs_s[et * 128:(et + 1) * 128, :], zt[:])

        # ---- depthwise causal conv + silu ----
        cw3 = cw_sb[:].rearrange("p (k c) -> p k c", k=KT)
        xc = pool.tile([128, KT * TOK], BF16, tag="xc")
        xc3 = xc[:].rearrange("p (k t) -> p k t", k=KT)
        for et in range(KT):
            a0 = pcv.tile([128, TOK], BF16, tag="cva")
            nc.vector.tensor_scalar_mul(a0[:], xin3[:, et, 1:1 + TOK],
                                        cw3[:, et, 0:1])
            cur = a0
            for j in range(1, KC):
                nxt = pcv.tile([128, TOK], BF16, tag=("cva" if j % 2 == 0 else "cvb"))
                nc.vector.scalar_tensor_tensor(
                    out=nxt[:], in0=xin3[:, et, j + 1:j + 1 + TOK],
                    scalar=cw3[:, et, j:j + 1], in1=cur[:],
                    op0=MULT, op1=ADD)
                cur = nxt
            nc.scalar.activation(xc3[:, et], cur[:], AF.Silu,
                                 bias=cb_sb[:, et:et + 1])
            nc.scalar.dma_start(xc_s[et * 128:(et + 1) * 128, :], xc3[:, et])

        # ---- x_proj -> dbl [96, TOK] ----
        wxp3 = wxp_sb[:].rearrange("p (k e) -> p k e", k=KT)
        ps_dbl = psb.tile([96, 1024], F32, tag="pstat0")
        for (o, w_) in [(0, 512), (512, 512)]:
            for kk in range(KT):
                nc.tensor.matmul(ps_dbl[:, o:o + w_], wxp3[:, kk],
                                 xc3[:, kk, o:o + w_],
                                 start=(kk == 0), stop=(kk == KT - 1))
        dbl = pool.tile([96, TOK], BF16, tag="dbl")
        nc.scalar.copy(dbl[:], ps_dbl[:])
        nc.sync.dma_start(bc_s[:], dbl[64:80, :])
        nc.sync.dma_start(cc_s[:], dbl[80:96, :])

        # ---- dt_proj + softplus ----
        for et in range(KT):
            ps = psb.tile([128, 1024], F32, tag=f"pstat{et % 2}")
            for (o, w_) in [(0, 512), (512, 512)]:
                nc.tensor.matmul(ps[:, o:o + w_],
                                 wdt_sb[:, et * 128:(et + 1) * 128],
                                 dbl[0:64, o:o + w_], start=True, stop=True)
            # softplus(u) = log1p(e^u) via series in t = e^{u+b} (t <= ~0.2):
            #   t - t^2/2 + t^3/3 - t^4/4
            tt = pcv.tile([128, TOK], F32, tag="spt")
            nc.scalar.activation(tt[:], ps[:], AF.Exp,
                                 bias=dtb_sb[:, et:et + 1])
            p2 = pcv.tile([128, TOK], F32, tag="spa")
            nc.vector.tensor_tensor(p2[:], tt[:], tt[:], op=MULT)
            dl = pev.tile([128, TOK], F32, tag="scratch")
            nc.vector.scalar_tensor_tensor(out=dl[:], in0=p2[:], scalar=-0.5,
                                           in1=tt[:], op0=MULT, op1=ADD)
            p3 = pcv.tile([128, TOK], F32, tag="spb")
            nc.vector.tensor_tensor(p3[:], p2[:], tt[:], op=MULT)
            nc.vector.scalar_tensor_tensor(out=dl[:], in0=p3[:],
                                           scalar=1.0 / 3.0, in1=dl[:],
                                           op0=MULT, op1=ADD)
            p4 = pcv.tile([128, TOK], F32, tag="spb")
            nc.vector.tensor_tensor(p4[:], p2[:], p2[:], op=MULT)
            nc.vector.scalar_tensor_tensor(out=dl[:], in0=p4[:], scalar=-0.25,
                                           in1=dl[:], op0=MULT, op1=ADD)
            nc.scalar.dma_start(delta_s[et * 128:(et + 1) * 128, :], dl[:])

    _fix_sync_waits(nc)
    return nc


# ---------------------------------------------------------------------------
def build_launch_b():
    nc = _new_nc()
    dl = nc.dram_tensor("dl", [ES, BT], F32, kind="ExternalInput").ap()
    xc = nc.dram_tensor("xc", [ES, BT], BF16, kind="ExternalInput").ap()
    zs = nc.dram_tensor("zs", [ES, BT], BF16, kind="ExternalInput").ap()
    bfl = nc.dram_tensor("bfl", [1, NCH * N * TC], BF16, kind="ExternalInput").ap()
    cfl = nc.dram_tensor("cfl", [1, NCH * N * TC], BF16, kind="ExternalInput").ap()
    aneg = nc.dram_tensor("aneg", [ES, N], F32, kind="ExternalInput").ap()
    dpr = nc.dram_tensor("dpr", [ES, 1], F32, kind="ExternalInput").ap()
    ys = nc.dram_tensor("ys", [ES, BT], BF16, kind="ExternalOutput").ap()

    with tile.TileContext(nc) as tc, ExitStack() as ctx:
        pool = ctx.enter_context(tc.tile_pool(name="p", bufs=1))
        pin = ctx.enter_context(tc.tile_pool(name="pin", bufs=2))
        prow = ctx.enter_context(tc.tile_pool(name="prow", bufs=1))
        pbig = ctx.enter_context(tc.tile_pool(name="pbig", bufs=1))
        pout = ctx.enter_context(tc.tile_pool(name="pout", bufs=2))
        pps = ctx.enter_context(tc.tile_pool(name="pps", bufs=2, space="PSUM"))

        ones_bf = pool.tile([1, 128], BF16, tag="ones_bf")
        nc.vector.memset(ones_bf[:], 1.0)
        an = pool.tile([ES, N], F32, tag="an")
        nc.sync.dma_start(an[:], aneg[:])
        dp = pool.tile([ES, 1], F32, tag="dp")
        nc.sync.dma_start(dp[:], dpr[:])
        hprev = pool.tile([ES, N], F32, tag="hprev")

        for ch in range(NCH):
            t0 = ch * TC
            dlc = pin.tile([ES, TC], F32, tag="dlc")
            nc.sync.dma_start(dlc[:], dl[:, t0:t0 + TC])
            xcc = pin.tile([ES, TC], BF16, tag="xcc")
            nc.sync.dma_start(xcc[:], xc[:, t0:t0 + TC])
            zsc = pin.tile([ES, TC], BF16, tag="zsc")
            nc.sync.dma_start(zsc[:], zs[:, t0:t0 + TC])
            brow = prow.tile([1, N * TC], BF16, tag="brow")
            nc.sync.dma_start(brow[:], bfl[:, ch * N * TC:(ch + 1) * N * TC])
            crow = prow.tile([1, N * TC], BF16, tag="crow")
            nc.sync.dma_start(crow[:], cfl[:, ch * N * TC:(ch + 1) * N * TC])

            # dA (f32, n-major) via ACT exp with per-partition scale A[:,n]
            dA = pbig.tile([ES, N * TC], F32, tag="dA")
            dA3 = dA[:].rearrange("p (n t) -> p n t", n=N)
            for n in range(N):
                nc.scalar.activation(dA3[:, n], dlc[:], AF.Exp,
                                     scale=an[:, n:n + 1])

            # w = delta*xc (bf16); dBx_n = w ⊙ B_n  (B_n copied psum->sbuf
            # bf16 on ScalarE so the DVE mult runs in 2x packed mode)
            wc = pin.tile([ES, TC], BF16, tag="wc")
            nc.vector.tensor_tensor(wc[:], dlc[:], xcc[:], op=MULT)
            dBx = pbig.tile([ES, N * TC], BF16, tag="dBx")
            dBx3 = dBx[:].rearrange("p (n t) -> p n t", n=N)
            for n in range(N):
                bps = pps.tile([ES, TC], F32, tag=f"bps{n % 2}")
                nc.tensor.matmul(bps[:], ones_bf[:],
                                 brow[:, n * TC:(n + 1) * TC],
                                 start=True, stop=True)
                bsb = pin.tile([ES, TC], BF16, tag=f"bsb{n % 2}")
                nc.scalar.copy(bsb[:], bps[:])
                nc.vector.tensor_tensor(dBx3[:, n], wc[:], bsb[:], op=MULT)

            # scans (fp32 internal state; bf16 out; initial = hprev or 0)
            hall = pbig.tile([ES, N * TC], BF16, tag="hall")
            hall3 = hall[:].rearrange("p (n t) -> p n t", n=N)
            for n in range(N):
                init = 0.0 if t0 % T == 0 else hprev[:, n:n + 1]
                nc.vector.tensor_tensor_scan(
                    hall3[:, n], dA3[:, n], dBx3[:, n], init,
                    op0=MULT, op1=ADD)
            # carry state: h[:, n, TC-1]
            nc.vector.tensor_copy(hprev[:], hall3[:, :, TC - 1])

            # hc = h ⊙ C (2x bf16) ; pairwise tree: 16 -> 8 -> 4 -> 2 -> 1
            hc = pbig.tile([ES, N * TC], BF16, tag="dBx")
            hc3 = hc[:].rearrange("p (n t) -> p n t", n=N)
            for n in range(N):
                cps = pps.tile([ES, TC], F32, tag=f"cps{n % 2}")
                nc.tensor.matmul(cps[:], ones_bf[:],
                                 crow[:, n * TC:(n + 1) * TC],
                                 start=True, stop=True)
                csb = pin.tile([ES, TC], BF16, tag=f"csb{n % 2}")
                nc.scalar.copy(csb[:], cps[:])
                nc.vector.tensor_tensor(hc3[:, n], hall3[:, n], csb[:],
                                        op=MULT)
            red = hc[:]
            width = N
            while width > 2:
                width //= 2
                nc.vector.tensor_tensor(
                    red[:, 0:width * TC], red[:, 0:width * TC],
                    red[:, width * TC:2 * width * TC], op=ADD)
            yv = pin.tile([ES, TC], F32, tag="yv")
            nc.vector.tensor_tensor(yv[:], red[:, 0:TC], red[:, TC:2 * TC],
                                    op=ADD)
            # y = (yv + xc*D) * zs
            nc.vector.scalar_tensor_tensor(
                out=yv[:], in0=xcc[:], scalar=dp[:, 0:1], in1=yv[:],
                op0=MULT, op1=ADD)
            yo = pout.tile([ES, TC], BF16, tag="yo")
            nc.vector.tensor_tensor(yo[:], yv[:], zsc[:], op=MULT)
            nc.scalar.dma_start(ys[:, t0:t0 + TC], yo[:])

    _fix_sync_waits(nc)
    return nc


# ---------------------------------------------------------------------------
def build_launch_c():
    nc = _new_nc()
    ysd = nc.dram_tensor("ysd", [E, TOK], BF16, kind="ExternalInput").ap()
    xsd = nc.dram_tensor("xsd", [D, TOK], F32, kind="ExternalInput").ap()
    wop = nc.dram_tensor("wop", [E, D], BF16, kind="ExternalInput").ap()
    w1t = nc.dram_tensor("w1t", [D, H], BF16, kind="ExternalInput").ap()
    w2t = nc.dram_tensor("w2t", [H, D], BF16, kind="ExternalInput").ap()
    b1f = nc.dram_tensor("b1f", [H, 1], F32, kind="ExternalInput").ap()
    b2 = nc.dram_tensor("b2", [D, 1], F32, kind="ExternalInput").ap()
    osd = nc.dram_tensor("osd", [D, TOK], F32, kind="ExternalOutput").ap()

    HP = [(0, 512), (512, 512)]

    with tile.TileContext(nc) as tc, ExitStack() as ctx:
        pool = ctx.enter_context(tc.tile_pool(name="p", bufs=1))
        pw = ctx.enter_context(tc.tile_pool(name="pw", bufs=2))
        pev = ctx.enter_context(tc.tile_pool(name="ev", bufs=2))
        psum = ctx.enter_context(tc.tile_pool(name="ps", bufs=1, space="PSUM"))
        psb = ctx.enter_context(tc.tile_pool(name="psb", bufs=2, space="PSUM"))

        ysb = pool.tile([128, KT * TOK], BF16, tag="tbig")
        nc.sync.dma_start(ysb[:].rearrange("p (k t) -> p k t", k=KT),
                          ysd[:].rearrange("(k p) t -> p k t", p=128))
        xsb = pool.tile([128, KT * TOK], F32, tag="xsb")
        nc.sync.dma_start(xsb[:].rearrange("p (k t) -> p k t", k=KT),
                          xsd[:].rearrange("(k p) t -> p k t", p=128))
        b2_sb = pool.tile([128, KT], F32, tag="b2")
        nc.sync.dma_start(b2_sb[:].unsqueeze(2),
                          b2[:].rearrange("(k p) o -> p k o", p=128))
        b1_sb = pool.tile([128, H // 128], F32, tag="b1")
        nc.sync.dma_start(b1_sb[:].unsqueeze(2),
                          b1f[:].rearrange("(k p) o -> p k o", p=128))
        ones_k = pool.tile([128, 1], F32, tag="ones_k")
        nc.vector.memset(ones_k[:], 1.0)
        ones_r = pool.tile([1, 128], F32, tag="ones_r")
        nc.vector.memset(ones_r[:], 1.0)

        ysb3 = ysb[:].rearrange("p (k t) -> p k t", k=KT)
        xsb3 = xsb[:].rearrange("p (k t) -> p k t", k=KT)

        # ---- x1 = x + out_proj(y)  (wop streamed) ----
        x1 = pool.tile([128, KT * TOK], F32, tag="x1")
        x13 = x1[:].rearrange("p (k t) -> p k t", k=KT)
        for m in range(KT):
            wm = pw.tile([128, KT * 128], BF16, tag="wm")
            nc.sync.dma_start(
                wm[:].rearrange("p (k e) -> p k e", k=KT),
                wop[:, m * 128:(m + 1) * 128].rearrange(
                    "(k p) e -> p k e", p=128))
            wm3 = wm[:].rearrange("p (k e) -> p k e", k=KT)
            ps = psb.tile([128, 1024], F32, tag="pmm")
            for (o, w_) in HP:
                for kk in range(KT):
                    nc.tensor.matmul(ps[:, o:o + w_], wm3[:, kk],
                                     ysb3[:, kk, o:o + w_],
                                     start=(kk == 0), stop=(kk == KT - 1))
            nc.vector.tensor_tensor(x13[:, m], xsb3[:, m], ps[:], op=ADD)

        # ---- LN2 (stats via ones-matmul; squares streamed) ----
        ps_x = psum.tile([1, 1024], F32, tag="pc0")
        ps_q = psum.tile([1, 1024], F32, tag="pc1")
        for kk in range(KT):
            sqt = pev.tile([128, TOK], F32, tag="s1")
            nc.scalar.activation(sqt[:], x13[:, kk], AF.Square)
            for (o, w_) in HP:
                nc.tensor.matmul(ps_x[:, o:o + w_], ones_k[:],
                                 x13[:, kk, o:o + w_],
                                 start=(kk == 0), stop=(kk == KT - 1))
            for (o, w_) in HP:
                nc.tensor.matmul(ps_q[:, o:o + w_], ones_k[:],
                                 sqt[:, o:o + w_],
                                 start=(kk == 0), stop=(kk == KT - 1))
        mu = pool.tile([1, TOK], F32, tag="mu")
        nc.scalar.mul(mu[:], ps_x[:], 1.0 / D)
        ex2 = pool.tile([1, TOK], F32, tag="ex2")
        nc.scalar.mul(ex2[:], ps_q[:], 1.0 / D)
        var = pool.tile([1, TOK], F32, tag="var")
        nc.vector.tensor_tensor(var[:], mu[:], mu[:], op=MULT)
        nc.vector.tensor_tensor(var[:], ex2[:], var[:], op=SUB)
        epst = pool.tile([1, 1], F32, tag="epst")
        nc.vector.memset(epst[:], EPS)
        std = pool.tile([1, TOK], F32, tag="std")
        nc.scalar.activation(std[:], var[:], AF.Sqrt, bias=epst[:])
        rstd = pool.tile([1, TOK], F32, tag="rstd")
        nc.vector.reciprocal(rstd[:], std[:])
        ms = pool.tile([1, TOK], F32, tag="ms")
        nc.vector.tensor_tensor(ms[:], mu[:], rstd[:], op=MULT)
        ps_rs = psum.tile([128, 1024], F32, tag="pc0")
        ps_ms = psum.tile([128, 1024], F32, tag="pc1")
        for (o, w_) in HP:
            nc.tensor.matmul(ps_rs[:, o:o + w_], ones_r[:], rstd[:, o:o + w_],
                             start=True, stop=True)
            nc.tensor.matmul(ps_ms[:, o:o + w_], ones_r[:], ms[:, o:o + w_],
                             start=True, stop=True)
        hl = pool.tile([128, KT * TOK], BF16, tag="tbig")
        hl3 = hl[:].rearrange("p (k t) -> p k t", k=KT)
        for kk in range(KT):
            t1 = pev.tile([128, TOK], F32, tag="s1")
            nc.vector.tensor_tensor(t1[:], x13[:, kk], ps_rs[:], op=MULT)
            nc.vector.tensor_tensor(hl3[:, kk], t1[:], ps_ms[:], op=SUB)

        # ---- FFN1: h1 = relu(W1.T @ hl + b1)  (bf16, split tiles;
        # h1a reuses the dead xsb slot) ----
        h1a = pool.tile([128, (H // 256) * TOK], BF16, tag="xsb")
        h1b = pool.tile([128, (H // 256) * TOK], BF16, tag="h1b")
        h1a3 = h1a[:].rearrange("p (k t) -> p k t", k=H // 256)
        h1b3 = h1b[:].rearrange("p (k t) -> p k t", k=H // 256)

        def h13(m):
            return h1a3[:, m] if m < H // 256 else h1b3[:, m - H // 256]

        for m in range(H // 128):
            wm = pw.tile([128, KT * 128], BF16, tag="wm")
            nc.sync.dma_start(
                wm[:].rearrange("p (k e) -> p k e", k=KT),
                w1t[:, m * 128:(m + 1) * 128].rearrange(
                    "(k p) e -> p k e", p=128))
            wm3 = wm[:].rearrange("p (k e) -> p k e", k=KT)
            ps = psb.tile([128, 1024], F32, tag="pmm")
            for (o, w_) in HP:
                for kk in range(KT):
                    nc.tensor.matmul(ps[:, o:o + w_], wm3[:, kk],
                                     hl3[:, kk, o:o + w_],
                                     start=(kk == 0), stop=(kk == KT - 1))
            nc.scalar.activation(h13(m), ps[:], AF.Relu,
                                 bias=b1_sb[:, m:m + 1])

        # ---- FFN2 + residual + b2 ----
        HK = H // 128
        for m in range(KT):
            ps = psb.tile([128, 1024], F32, tag="pmm")
            QK = HK // 4
            for q in range(4):
                wm = pw.tile([128, QK * 128], BF16, tag="wm2")
                nc.sync.dma_start(
                    wm[:].rearrange("p (k e) -> p k e", k=QK),
                    w2t[q * (H // 4):(q + 1) * (H // 4),
                        m * 128:(m + 1) * 128].rearrange(
                        "(k p) e -> p k e", p=128))
                wm3 = wm[:].rearrange("p (k e) -> p k e", k=QK)
                for (o, w_) in HP:
                    for kk in range(QK):
                        gk = q * QK + kk
                        nc.tensor.matmul(ps[:, o:o + w_], wm3[:, kk],
                                         h13(gk)[:, o:o + w_],
                                         start=(gk == 0), stop=(gk == HK - 1))
            t2 = pev.tile([128, TOK], F32, tag="s1")
            nc.scalar.activation(t2[:], ps[:], AF.Identity,
                                 bias=b2_sb[:, m:m + 1])
            oo = pev.tile([128, TOK], F32, tag="s2")
            nc.vector.tensor_tensor(oo[:], x13[:, m], t2[:], op=ADD)
            nc.scalar.dma_start(osd[m * 128:(m + 1) * 128, :], oo[:])

    _fix_sync_waits(nc)
    return nc


# ---------------------------------------------------------------------------
def _get_programs():
    if "progs" not in _cache:
        _cache["progs"] = (build_launch_a(), build_launch_b(), build_launch_c())
    return _cache["progs"]


def _launches():
    """(name, program, in_maps) for each launch of the last kernel() call."""
    progs = _get_programs()
    return [("A", progs[0], _cache["maps_a"]),
            ("B", progs[1], _cache["maps_b"]),
            ("C", progs[2], _cache["maps_c"])]


def kernel(**inputs) -> np.ndarray:
    x = np.ascontiguousarray(np.asarray(inputs["x"], np.float32))
    in_proj_w = np.asarray(inputs["in_proj_w"], np.float32)
    conv_w = np.asarray(inputs["conv_w"], np.float32)
    conv_b = np.asarray(inputs["conv_b"], np.float32)
    x_proj_w = np.asarray(inputs["x_proj_w"], np.float32)
    dt_proj_w = np.asarray(inputs["dt_proj_w"], np.float32)
    dt_proj_b = np.asarray(inputs["dt_proj_b"], np.float32)
    A_log = np.asarray(inputs["A_log"], np.float32)
    D_param = np.asarray(inputs["D_param"], np.float32)
    out_proj_w = np.asarray(inputs["out_proj_w"], np.float32)
    ln1_g = np.asarray(inputs["ln1_g"], np.float32)
    ln1_b = np.asarray(inputs["ln1_b"], np.float32)
    ln2_g = np.asarray(inputs["ln2_g"], np.float32)
    ln2_b = np.asarray(inputs["ln2_b"], np.float32)
    w1 = np.asarray(inputs["w1"], np.float32)
    b1 = np.asarray(inputs["b1"], np.float32)
    w2 = np.asarray(inputs["w2"], np.float32)
    b2v = np.asarray(inputs["b2"], np.float32)

    nca, ncb, ncc = _get_programs()

    # ---- host prep ----
    xT = np.ascontiguousarray(x.reshape(BT, D).T)              # [D, BT]
    WxzT = (in_proj_w * ln1_g[None, :]).T.astype(ml_dtypes.bfloat16)
    bias_xz = (in_proj_w @ ln1_b).reshape(2 * E, 1)
    wxpT = x_proj_w.T.astype(ml_dtypes.bfloat16)
    wdtT = dt_proj_w.T.astype(ml_dtypes.bfloat16)
    Aneg = -np.exp(A_log)
    wopT = out_proj_w.T.astype(ml_dtypes.bfloat16)
    W1T = (w1 * ln2_g[None, :]).T.astype(ml_dtypes.bfloat16)
    b1fv = (b1 + w1 @ ln2_b).reshape(H, 1)
    W2Tb = np.ascontiguousarray(w2.T).astype(ml_dtypes.bfloat16)

    # ---- launch A ----
    in_maps_a = []
    for c in range(NC):
        lo, hi = c * TOK, (c + 1) * TOK
        xs_c = np.zeros((D, TOKH), np.float32)
        batch0 = (lo // T) * T
        hlo = max(lo - HALO, batch0)
        nh = lo - hlo
        if nh:
            xs_c[:, HALO - nh:HALO] = xT[:, hlo:lo]
        xs_c[:, HALO:] = xT[:, lo:hi]
        in_maps_a.append(dict(
            xs=xs_c, wxz=WxzT, wxp=wxpT, wdt=wdtT, cw=conv_w,
            cbias=conv_b.reshape(E, 1), dtb=dt_proj_b.reshape(E, 1),
            bxz=bias_xz))
    _cache['maps_a'] = in_maps_a
    ra = run_bass_kernel_spmd(nca, in_maps_a, core_ids=list(range(NC)))

    delta_g = np.concatenate([r["delta_s"] for r in ra.results], axis=1)
    xc_g = np.concatenate([r["xc_s"] for r in ra.results], axis=1)
    zs_g = np.concatenate([r["zs_s"] for r in ra.results], axis=1)
    Bg = np.concatenate([r["bc_s"] for r in ra.results], axis=1)  # bf16
    Cg = np.concatenate([r["cc_s"] for r in ra.results], axis=1)

    # n-major per chunk: [NCH, N, TC]
    bfl = np.ascontiguousarray(
        Bg.reshape(N, NCH, TC).transpose(1, 0, 2)).reshape(1, -1)
    cfl = np.ascontiguousarray(
        Cg.reshape(N, NCH, TC).transpose(1, 0, 2)).reshape(1, -1)

    in_maps_b = []
    for c in range(NC):
        elo, ehi = c * ES, (c + 1) * ES
        in_maps_b.append(dict(
            dl=np.ascontiguousarray(delta_g[elo:ehi]),
            xc=np.ascontiguousarray(xc_g[elo:ehi]),
            zs=np.ascontiguousarray(zs_g[elo:ehi]),
            bfl=bfl, cfl=cfl,
            aneg=np.ascontiguousarray(Aneg[elo:ehi]),
            dpr=np.ascontiguousarray(D_param[elo:ehi].reshape(ES, 1))))
    _cache['maps_b'] = in_maps_b
    rb = run_bass_kernel_spmd(ncb, in_maps_b, core_ids=list(range(NC)))
    y_g = np.concatenate([r["ys"] for r in rb.results], axis=0)  # [E, BT]

    in_maps_c = []
    for c in range(NC):
        lo, hi = c * TOK, (c + 1) * TOK
        in_maps_c.append(dict(
            ysd=np.ascontiguousarray(y_g[:, lo:hi]),
            xsd=np.ascontiguousarray(xT[:, lo:hi]),
            wop=wopT, w1t=W1T, w2t=W2Tb, b1f=b1fv,
            b2=b2v.reshape(D, 1)))
    _cache['maps_c'] = in_maps_c
    rc = run_bass_kernel_spmd(ncc, in_maps_c, core_ids=list(range(NC)))
    out_cm = np.concatenate([r["osd"] for r in rc.results], axis=1)  # [D, BT]

    return np.ascontiguousarray(out_cm.T).reshape(B, T, D).astype(np.float32)



# revision 29
# speedup vs baseline: 1.0383x; 1.0383x over previous
"""Mamba block + FFN on 8 Trainium2 NeuronCores (Bass/Tile, SPMD).

Self-contained: hardcoded shapes for nn_Block_472446402621.
  x [2, 4096, 1024] fp32 -> out [2, 4096, 1024] fp32

Three SPMD launches with host-side (free) reshuffling between:
  A: token-parallel  — LN1 + in_proj + causal conv + silu + x_proj + dt_proj
  B: channel-parallel — selective scan (tensor_tensor_scan) + gating
  C: token-parallel  — out_proj + residual + LN2 + FFN + residual
"""
import numpy as np
import ml_dtypes
from contextlib import ExitStack

import concourse.bass as bass
import concourse.tile as tile
from concourse import mybir
from concourse.bass_utils import run_bass_kernel_spmd

F32 = mybir.dt.float32
F32R = mybir.dt.float32r
BF16 = mybir.dt.bfloat16
AF = mybir.ActivationFunctionType
MULT = mybir.AluOpType.mult
ADD = mybir.AluOpType.add
SUB = mybir.AluOpType.subtract
AXX = mybir.AxisListType.X

B, T, D = 2, 4096, 1024
E, N, RK, H, KC = 1024, 16, 64, 4096, 8
BT = B * T
NC = 8
TOK = BT // NC          # tokens per core (A, C)
HALO = 8                # conv halo columns (first 8 of xs)
TOKH = TOK + HALO
ES = E // NC            # channels per core (B)
TC = 512                # scan chunk length
NCH = BT // TC
KT = D // 128           # k tiles over d_model
EPS = 1e-5

_cache = {}


# ---------------------------------------------------------------------------
# sync-wait legalization: walrus instruction encodings hold ~1 sem wait;
# hoist excess monotone waits into standalone EventSemaphore instructions.
def _fix_sync_waits(nc):
    cnt = 0
    for f in nc.m.functions:
        for bb in f.blocks:
            insts = bb.instructions
            out = []
            changed = False
            for inst in insts:
                si = inst.sync_info
                waits = list(si.on_wait) if si is not None else []
                if len(waits) > 1:
                    hoist, keep = [], []
                    for w in waits:
                        if w.wait_mode in ("sem-ge-imm", "sem-ge-reg"):
                            hoist.append(w)
                        else:
                            keep.append(w)
                    room = 1 - len(keep)
                    assert room >= 0, f"{inst.name}: non-monotone waits > 1"
                    keep += hoist[:room]
                    hoist = hoist[room:]
                    for w in hoist:
                        cnt += 1
                        ni = mybir.InstEventSemaphore(
                            name=f"W-fix-{cnt}", ins=[], outs=[],
                            sync_info=mybir.SyncInfo(on_wait=[w], on_update=[]))
                        ni.engine = inst.engine
                        out.append(ni)
                    inst.sync_info = mybir.SyncInfo(
                        on_wait=keep, on_update=list(si.on_update))
                    changed = True
                out.append(inst)
            if changed:
                bb.instructions = out
    return cnt


def _new_nc():
    return bass.Bass("TRN2", target_bir_lowering=False, debug=False,
                     num_devices=NC)


# ---------------------------------------------------------------------------
def build_launch_a():
    nc = _new_nc()
    xs = nc.dram_tensor("xs", [D, TOKH], F32, kind="ExternalInput").ap()
    wxz = nc.dram_tensor("wxz", [D, 2 * E], BF16, kind="ExternalInput").ap()
    wxp = nc.dram_tensor("wxp", [E, 96], BF16, kind="ExternalInput").ap()
    wdt = nc.dram_tensor("wdt", [RK, E], BF16, kind="ExternalInput").ap()
    cw = nc.dram_tensor("cw", [E, KC], F32, kind="ExternalInput").ap()
    cbias = nc.dram_tensor("cbias", [E, 1], F32, kind="ExternalInput").ap()
    dtb = nc.dram_tensor("dtb", [E, 1], F32, kind="ExternalInput").ap()
    bxz = nc.dram_tensor("bxz", [2 * E, 1], F32, kind="ExternalInput").ap()

    delta_s = nc.dram_tensor("delta_s", [E, TOK], F32, kind="ExternalOutput").ap()
    xc_s = nc.dram_tensor("xc_s", [E, TOK], BF16, kind="ExternalOutput").ap()
    zs_s = nc.dram_tensor("zs_s", [E, TOK], BF16, kind="ExternalOutput").ap()
    bc_s = nc.dram_tensor("bc_s", [N, TOK], BF16, kind="ExternalOutput").ap()
    cc_s = nc.dram_tensor("cc_s", [N, TOK], BF16, kind="ExternalOutput").ap()

    with tile.TileContext(nc) as tc, ExitStack() as ctx:
        pool = ctx.enter_context(tc.tile_pool(name="p", bufs=1))
        pev = ctx.enter_context(tc.tile_pool(name="ev", bufs=2))
        pcv = ctx.enter_context(tc.tile_pool(name="cv", bufs=1))
        pw = ctx.enter_context(tc.tile_pool(name="pw", bufs=3))
        psum = ctx.enter_context(tc.tile_pool(name="ps", bufs=1, space="PSUM"))
        psb = psum

        # ---- load x (8 k-tiles) ----
        xk = pool.tile([128, KT * TOKH], F32, tag="xk")
        nc.sync.dma_start(
            xk[:].rearrange("p (k t) -> p k t", k=KT),
            xs[:].rearrange("(k p) t -> p k t", p=128))
        # weights (wxz streamed per output e-tile later)
        wxp_sb = pool.tile([128, KT * 96], BF16, tag="wxp")
        nc.sync.dma_start(
            wxp_sb[:].rearrange("p (k e) -> p k e", k=KT),
            wxp[:].rearrange("(k p) e -> p k e", p=128))
        wdt_sb = pool.tile([RK, E], BF16, tag="wdt")
        nc.sync.dma_start(wdt_sb[:], wdt[:])
        cw_sb = pool.tile([128, KT * KC], F32, tag="cw")
        nc.sync.dma_start(
            cw_sb[:].rearrange("p (k c) -> p k c", k=KT),
            cw[:].rearrange("(k p) c -> p k c", p=128))
        cb_sb = pool.tile([128, KT], F32, tag="cb")
        nc.sync.dma_start(cb_sb[:].unsqueeze(2),
                          cbias[:].rearrange("(k p) o -> p k o", p=128))
        dtb_sb = pool.tile([128, KT], F32, tag="dtb")
        nc.sync.dma_start(dtb_sb[:].unsqueeze(2),
                          dtb[:].rearrange("(k p) o -> p k o", p=128))
        bxz_sb = pool.tile([128, 2 * KT], F32, tag="bxz")
        nc.sync.dma_start(bxz_sb[:].unsqueeze(2),
                          bxz[:].rearrange("(k p) o -> p k o", p=128))

        ones_k = pool.tile([128, 1], F32, tag="ones_k")
        nc.vector.memset(ones_k[:], 1.0)
        ones_r = pool.tile([1, 128], F32, tag="ones_r")
        nc.vector.memset(ones_r[:], 1.0)

        xk3 = xk[:].rearrange("p (k t) -> p k t", k=KT)

        # ---- LN1 stats (square streamed per k-tile) ----
        ps_x = psum.tile([1, 1536], F32, tag="pstat0")
        ps_q = psum.tile([1, 1536], F32, tag="pstat1")
        pieces = [(0, 512), (512, 512), (1024, HALO)]
        for kk in range(KT):
            sqt = pev.tile([128, TOKH], F32, tag="scratch")
            nc.scalar.activation(sqt[:], xk3[:, kk], AF.Square)
            for (o, w_) in pieces:
                nc.tensor.matmul(ps_x[:, o:o + w_], ones_k[:],
                                 xk3[:, kk, o:o + w_],
                                 start=(kk == 0), stop=(kk == KT - 1))
            for (o, w_) in pieces:
                nc.tensor.matmul(ps_q[:, o:o + w_], ones_k[:],
                                 sqt[:, o:o + w_],
                                 start=(kk == 0), stop=(kk == KT - 1))

        mu = pool.tile([1, TOKH], F32, tag="mu")
        nc.scalar.mul(mu[:], ps_x[:, 0:TOKH], 1.0 / D)
        ex2 = pool.tile([1, TOKH], F32, tag="ex2")
        nc.scalar.mul(ex2[:], ps_q[:, 0:TOKH], 1.0 / D)
        var = pool.tile([1, TOKH], F32, tag="var")
        nc.vector.tensor_tensor(var[:], mu[:], mu[:], op=MULT)
        nc.vector.tensor_tensor(var[:], ex2[:], var[:], op=SUB)
        epst = pool.tile([1, 1], F32, tag="epst")
        nc.vector.memset(epst[:], EPS)
        std = pool.tile([1, TOKH], F32, tag="std")
        nc.scalar.activation(std[:], var[:], AF.Sqrt, bias=epst[:])
        rstd = pool.tile([1, TOKH], F32, tag="rstd")
        nc.vector.reciprocal(rstd[:], std[:])
        ms = pool.tile([1, TOKH], F32, tag="ms")
        nc.vector.tensor_tensor(ms[:], mu[:], rstd[:], op=MULT)

        ps_rs = psum.tile([128, 1536], F32, tag="pstat0")
        ps_ms = psum.tile([128, 1536], F32, tag="pstat1")
        for (o, w_) in pieces:
            nc.tensor.matmul(ps_rs[:, o:o + w_], ones_r[:], rstd[:, o:o + w_],
                             start=True, stop=True)
            nc.tensor.matmul(ps_ms[:, o:o + w_], ones_r[:], ms[:, o:o + w_],
                             start=True, stop=True)

        # ---- xhat = x*rstd - mu*rstd   (bf16) ----
        xhat = pool.tile([128, KT * TOKH], BF16, tag="xhat")
        xhat3 = xhat[:].rearrange("p (k t) -> p k t", k=KT)
        tmp = pev.tile([128, TOKH], F32, tag="lntmp")
        for kk in range(KT):
            tmp = pev.tile([128, TOKH], F32, tag="lntmp")
            nc.vector.tensor_tensor(tmp[:], xk3[:, kk], ps_rs[:, 0:TOKH], op=MULT)
            nc.vector.tensor_tensor(xhat3[:, kk], tmp[:], ps_ms[:, 0:TOKH], op=SUB)

        # ---- in_proj: xin (e-tiles 0..7) and z (8..15), wxz streamed ----
        xin = pool.tile([128, KT * TOKH], BF16, tag="xin")
        xin3 = xin[:].rearrange("p (k t) -> p k t", k=KT)
        for et in range(KT):
            wm = pw.tile([128, KT * 128], BF16, tag="wmA")
            nc.sync.dma_start(
                wm[:].rearrange("p (k e) -> p k e", k=KT),
                wxz[:, et * 128:(et + 1) * 128].rearrange(
                    "(k p) e -> p k e", p=128))
            wm3 = wm[:].rearrange("p (k e) -> p k e", k=KT)
            ps = psb.tile([128, 1536], F32, tag=f"pstat{et % 2}")
            for (o, w_) in pieces:
                for kk in range(KT):
                    nc.tensor.matmul(
                        ps[:, o:o + w_], wm3[:, kk],
                        xhat3[:, kk, o:o + w_],
                        start=(kk == 0), stop=(kk == KT - 1))
            nc.scalar.activation(xin3[:, et], ps[:, 0:TOKH], AF.Identity,
                                 bias=bxz_sb[:, et:et + 1])
        for et in range(KT):
            wm = pw.tile([128, KT * 128], BF16, tag="wmA")
            nc.sync.dma_start(
                wm[:].rearrange("p (k e) -> p k e", k=KT),
                wxz[:, E + et * 128:E + (et + 1) * 128].rearrange(
                    "(k p) e -> p k e", p=128))
            wm3 = wm[:].rearrange("p (k e) -> p k e", k=KT)
            ps = psb.tile([128, 1536], F32, tag=f"pstat{et % 2}")
            for (o, w_) in pieces:
                for kk in range(KT):
                    nc.tensor.matmul(
                        ps[:, o:o + w_], wm3[:, kk],
                        xhat3[:, kk, o:o + w_],
                        start=(kk == 0), stop=(kk == KT - 1))
            zt = pev.tile([128, TOK], BF16, tag="zbf")
            nc.scalar.activation(zt[:], ps[:, HALO:TOKH], AF.Silu,
                                 bias=bxz_sb[:, KT + et:KT + et + 1])
            nc.scalar.dma_start(zs_s[et * 128:(et + 1) * 128, :], zt[:])

        # ---- depthwise causal conv + silu ----
        cw3 = cw_sb[:].rearrange("p (k c) -> p k c", k=KT)
        xc = pool.tile([128, KT * TOK], BF16, tag="xc")
        xc3 = xc[:].rearrange("p (k t) -> p k t", k=KT)
        for et in range(KT):
            a0 = pcv.tile([128, TOK], BF16, tag="cva")
            nc.vector.tensor_scalar_mul(a0[:], xin3[:, et, 1:1 + TOK],
                                        cw3[:, et, 0:1])
            cur = a0
            for j in range(1, KC):
                nxt = pcv.tile([128, TOK], BF16, tag=("cva" if j % 2 == 0 else "cvb"))
                nc.vector.scalar_tensor_tensor(
                    out=nxt[:], in0=xin3[:, et, j + 1:j + 1 + TOK],
                    scalar=cw3[:, et, j:j + 1], in1=cur[:],
                    op0=MULT, op1=ADD)
                cur = nxt
            nc.scalar.activation(xc3[:, et], cur[:], AF.Silu,
                                 bias=cb_sb[:, et:et + 1])
            nc.scalar.dma_start(xc_s[et * 128:(et + 1) * 128, :], xc3[:, et])

        # ---- x_proj -> dbl [96, TOK] ----
        wxp3 = wxp_sb[:].rearrange("p (k e) -> p k e", k=KT)
        ps_dbl = psb.tile([96, 1024], F32, tag="pstat0")
        for (o, w_) in [(0, 512), (512, 512)]:
            for kk in range(KT):
                nc.tensor.matmul(ps_dbl[:, o:o + w_], wxp3[:, kk],
                                 xc3[:, kk, o:o + w_],
                                 start=(kk == 0), stop=(kk == KT - 1))
        dbl = pool.tile([96, TOK], BF16, tag="dbl")
        nc.scalar.copy(dbl[:], ps_dbl[:])
        nc.sync.dma_start(bc_s[:], dbl[64:80, :])
        nc.sync.dma_start(cc_s[:], dbl[80:96, :])

        # ---- dt_proj + softplus ----
        for et in range(KT):
            ps = psb.tile([128, 1024], F32, tag=f"pstat{et % 2}")
            for (o, w_) in [(0, 512), (512, 512)]:
                nc.tensor.matmul(ps[:, o:o + w_],
                                 wdt_sb[:, et * 128:(et + 1) * 128],
                                 dbl[0:64, o:o + w_], start=True, stop=True)
            # softplus(u) = log1p(e^u) via series in t = e^{u+b} (t <= ~0.2):
            #   t - t^2/2 + t^3/3 - t^4/4
            tt = pcv.tile([128, TOK], F32, tag="spt")
            nc.scalar.activation(tt[:], ps[:], AF.Exp,
                                 bias=dtb_sb[:, et:et + 1])
            p2 = pcv.tile([128, TOK], F32, tag="spa")
            nc.vector.tensor_tensor(p2[:], tt[:], tt[:], op=MULT)
            dl = pev.tile([128, TOK], F32, tag="scratch")
            nc.vector.scalar_tensor_tensor(out=dl[:], in0=p2[:], scalar=-0.5,
                                           in1=tt[:], op0=MULT, op1=ADD)
            p3 = pcv.tile([128, TOK], F32, tag="spb")
            nc.vector.tensor_tensor(p3[:], p2[:], tt[:], op=MULT)
            nc.vector.scalar_tensor_tensor(out=dl[:], in0=p3[:],
                                           scalar=1.0 / 3.0, in1=dl[:],
                                           op0=MULT, op1=ADD)
            p4 = pcv.tile([128, TOK], F32, tag="spb")
            nc.vector.tensor_tensor(p4[:], p2[:], p2[:], op=MULT)
            nc.vector.scalar_tensor_tensor(out=dl[:], in0=p4[:], scalar=-0.25,
                                           in1=dl[:], op0=MULT, op1=ADD)
            nc.scalar.dma_start(delta_s[et * 128:(et + 1) * 128, :], dl[:])

    _fix_sync_waits(nc)
    return nc


# ---------------------------------------------------------------------------
def build_launch_b():
    nc = _new_nc()
    dl = nc.dram_tensor("dl", [ES, BT], F32, kind="ExternalInput").ap()
    xc = nc.dram_tensor("xc", [ES, BT], BF16, kind="ExternalInput").ap()
    zs = nc.dram_tensor("zs", [ES, BT], BF16, kind="ExternalInput").ap()
    bfl = nc.dram_tensor("bfl", [1, NCH * N * TC], BF16, kind="ExternalInput").ap()
    cfl = nc.dram_tensor("cfl", [1, NCH * N * TC], BF16, kind="ExternalInput").ap()
    aneg = nc.dram_tensor("aneg", [ES, N], F32, kind="ExternalInput").ap()
    dpr = nc.dram_tensor("dpr", [ES, 1], F32, kind="ExternalInput").ap()
    ys = nc.dram_tensor("ys", [ES, BT], BF16, kind="ExternalOutput").ap()

    with tile.TileContext(nc) as tc, ExitStack() as ctx:
        pool = ctx.enter_context(tc.tile_pool(name="p", bufs=1))
        pin = ctx.enter_context(tc.tile_pool(name="pin", bufs=2))
        prow = ctx.enter_context(tc.tile_pool(name="prow", bufs=1))
        pbig = ctx.enter_context(tc.tile_pool(name="pbig", bufs=1))
        pout = ctx.enter_context(tc.tile_pool(name="pout", bufs=2))
        pps = ctx.enter_context(tc.tile_pool(name="pps", bufs=2, space="PSUM"))

        ones_bf = pool.tile([1, 128], BF16, tag="ones_bf")
        nc.vector.memset(ones_bf[:], 1.0)
        an = pool.tile([ES, N], F32, tag="an")
        nc.sync.dma_start(an[:], aneg[:])
        dp = pool.tile([ES, 1], F32, tag="dp")
        nc.sync.dma_start(dp[:], dpr[:])
        hprev = pool.tile([ES, N], F32, tag="hprev")

        for ch in range(NCH):
            t0 = ch * TC
            dlc = pin.tile([ES, TC], F32, tag="dlc")
            nc.sync.dma_start(dlc[:], dl[:, t0:t0 + TC])
            xcc = pin.tile([ES, TC], BF16, tag="xcc")
            nc.sync.dma_start(xcc[:], xc[:, t0:t0 + TC])
            zsc = pin.tile([ES, TC], BF16, tag="zsc")
            nc.sync.dma_start(zsc[:], zs[:, t0:t0 + TC])
            brow = prow.tile([1, N * TC], BF16, tag="brow")
            nc.sync.dma_start(brow[:], bfl[:, ch * N * TC:(ch + 1) * N * TC])
            crow = prow.tile([1, N * TC], BF16, tag="crow")
            nc.sync.dma_start(crow[:], cfl[:, ch * N * TC:(ch + 1) * N * TC])

            # dA (f32, n-major) via ACT exp with per-partition scale A[:,n]
            dA = pbig.tile([ES, N * TC], F32, tag="dA")
            dA3 = dA[:].rearrange("p (n t) -> p n t", n=N)
            for n in range(N):
                nc.scalar.activation(dA3[:, n], dlc[:], AF.Exp,
                                     scale=an[:, n:n + 1])

            # w = delta*xc (bf16); dBx_n = w ⊙ B_n
            wc = pin.tile([ES, TC], BF16, tag="wc")
            nc.vector.tensor_tensor(wc[:], dlc[:], xcc[:], op=MULT)
            dBx = pbig.tile([ES, N * TC], BF16, tag="dBx")
            dBx3 = dBx[:].rearrange("p (n t) -> p n t", n=N)
            for n in range(N):
                bps = pps.tile([ES, TC], F32, tag=f"bps{n % 2}")
                nc.tensor.matmul(bps[:], ones_bf[:],
                                 brow[:, n * TC:(n + 1) * TC],
                                 start=True, stop=True)
                nc.vector.tensor_tensor(dBx3[:, n], wc[:], bps[:], op=MULT)

            # scans (fp32 state; initial = hprev or 0 at batch starts)
            hall = pbig.tile([ES, N * TC], F32, tag="hall")
            hall3 = hall[:].rearrange("p (n t) -> p n t", n=N)
            for n in range(N):
                init = 0.0 if t0 % T == 0 else hprev[:, n:n + 1]
                nc.vector.tensor_tensor_scan(
                    hall3[:, n], dA3[:, n], dBx3[:, n], init,
                    op0=MULT, op1=ADD)
            # carry state: h[:, n, TC-1]
            nc.vector.tensor_copy(hprev[:], hall3[:, :, TC - 1])

            # hc = h ⊙ C ; pairwise tree: 16 -> 8 -> 4 -> 2 -> 1
            hc = pbig.tile([ES, N * TC], BF16, tag="dBx")
            hc3 = hc[:].rearrange("p (n t) -> p n t", n=N)
            for n in range(N):
                cps = pps.tile([ES, TC], F32, tag=f"cps{n % 2}")
                nc.tensor.matmul(cps[:], ones_bf[:],
                                 crow[:, n * TC:(n + 1) * TC],
                                 start=True, stop=True)
                nc.vector.tensor_tensor(hc3[:, n], hall3[:, n], cps[:],
                                        op=MULT)
            # tree runs on GpSimd (SBUF-only bf16 adds) to offload the
            # critical-path Vector engine
            red = hc[:]
            width = N
            while width > 2:
                width //= 2
                nc.gpsimd.tensor_tensor(
                    red[:, 0:width * TC], red[:, 0:width * TC],
                    red[:, width * TC:2 * width * TC], op=ADD)
            yv = pin.tile([ES, TC], F32, tag="yv")
            nc.gpsimd.tensor_tensor(yv[:], red[:, 0:TC], red[:, TC:2 * TC],
                                    op=ADD)
            # y = (yv + xc*D) * zs
            nc.vector.scalar_tensor_tensor(
                out=yv[:], in0=xcc[:], scalar=dp[:, 0:1], in1=yv[:],
                op0=MULT, op1=ADD)
            yo = pout.tile([ES, TC], BF16, tag="yo")
            nc.vector.tensor_tensor(yo[:], yv[:], zsc[:], op=MULT)
            nc.scalar.dma_start(ys[:, t0:t0 + TC], yo[:])

    _fix_sync_waits(nc)
    return nc


# ---------------------------------------------------------------------------
def build_launch_c():
    nc = _new_nc()
    ysd = nc.dram_tensor("ysd", [E, TOK], BF16, kind="ExternalInput").ap()
    xsd = nc.dram_tensor("xsd", [D, TOK], F32, kind="ExternalInput").ap()
    wop = nc.dram_tensor("wop", [E, D], BF16, kind="ExternalInput").ap()
    w1t = nc.dram_tensor("w1t", [D, H], BF16, kind="ExternalInput").ap()
    w2t = nc.dram_tensor("w2t", [H, D], BF16, kind="ExternalInput").ap()
    b1f = nc.dram_tensor("b1f", [H, 1], F32, kind="ExternalInput").ap()
    b2 = nc.dram_tensor("b2", [D, 1], F32, kind="ExternalInput").ap()
    osd = nc.dram_tensor("osd", [D, TOK], F32, kind="ExternalOutput").ap()

    HP = [(0, 512), (512, 512)]

    with tile.TileContext(nc) as tc, ExitStack() as ctx:
        pool = ctx.enter_context(tc.tile_pool(name="p", bufs=1))
        pw = ctx.enter_context(tc.tile_pool(name="pw", bufs=2))
        pev = ctx.enter_context(tc.tile_pool(name="ev", bufs=2))
        psum = ctx.enter_context(tc.tile_pool(name="ps", bufs=1, space="PSUM"))
        psb = ctx.enter_context(tc.tile_pool(name="psb", bufs=2, space="PSUM"))

        ysb = pool.tile([128, KT * TOK], BF16, tag="tbig")
        nc.sync.dma_start(ysb[:].rearrange("p (k t) -> p k t", k=KT),
                          ysd[:].rearrange("(k p) t -> p k t", p=128))
        xsb = pool.tile([128, KT * TOK], F32, tag="xsb")
        nc.sync.dma_start(xsb[:].rearrange("p (k t) -> p k t", k=KT),
                          xsd[:].rearrange("(k p) t -> p k t", p=128))
        b2_sb = pool.tile([128, KT], F32, tag="b2")
        nc.sync.dma_start(b2_sb[:].unsqueeze(2),
                          b2[:].rearrange("(k p) o -> p k o", p=128))
        b1_sb = pool.tile([128, H // 128], F32, tag="b1")
        nc.sync.dma_start(b1_sb[:].unsqueeze(2),
                          b1f[:].rearrange("(k p) o -> p k o", p=128))
        ones_k = pool.tile([128, 1], F32, tag="ones_k")
        nc.vector.memset(ones_k[:], 1.0)
        ones_r = pool.tile([1, 128], F32, tag="ones_r")
        nc.vector.memset(ones_r[:], 1.0)

        ysb3 = ysb[:].rearrange("p (k t) -> p k t", k=KT)
        xsb3 = xsb[:].rearrange("p (k t) -> p k t", k=KT)

        # ---- x1 = x + out_proj(y)  (wop streamed) ----
        x1 = pool.tile([128, KT * TOK], F32, tag="x1")
        x13 = x1[:].rearrange("p (k t) -> p k t", k=KT)
        for m in range(KT):
            wm = pw.tile([128, KT * 128], BF16, tag="wm")
            nc.sync.dma_start(
                wm[:].rearrange("p (k e) -> p k e", k=KT),
                wop[:, m * 128:(m + 1) * 128].rearrange(
                    "(k p) e -> p k e", p=128))
            wm3 = wm[:].rearrange("p (k e) -> p k e", k=KT)
            ps = psb.tile([128, 1024], F32, tag="pmm")
            for (o, w_) in HP:
                for kk in range(KT):
                    nc.tensor.matmul(ps[:, o:o + w_], wm3[:, kk],
                                     ysb3[:, kk, o:o + w_],
                                     start=(kk == 0), stop=(kk == KT - 1))
            nc.vector.tensor_tensor(x13[:, m], xsb3[:, m], ps[:], op=ADD)

        # ---- LN2 (stats via ones-matmul; squares streamed) ----
        ps_x = psum.tile([1, 1024], F32, tag="pc0")
        ps_q = psum.tile([1, 1024], F32, tag="pc1")
        for kk in range(KT):
            sqt = pev.tile([128, TOK], F32, tag="s1")
            nc.scalar.activation(sqt[:], x13[:, kk], AF.Square)
            for (o, w_) in HP:
                nc.tensor.matmul(ps_x[:, o:o + w_], ones_k[:],
                                 x13[:, kk, o:o + w_],
                                 start=(kk == 0), stop=(kk == KT - 1))
            for (o, w_) in HP:
                nc.tensor.matmul(ps_q[:, o:o + w_], ones_k[:],
                                 sqt[:, o:o + w_],
                                 start=(kk == 0), stop=(kk == KT - 1))
        mu = pool.tile([1, TOK], F32, tag="mu")
        nc.scalar.mul(mu[:], ps_x[:], 1.0 / D)
        ex2 = pool.tile([1, TOK], F32, tag="ex2")
        nc.scalar.mul(ex2[:], ps_q[:], 1.0 / D)
        var = pool.tile([1, TOK], F32, tag="var")
        nc.vector.tensor_tensor(var[:], mu[:], mu[:], op=MULT)
        nc.vector.tensor_tensor(var[:], ex2[:], var[:], op=SUB)
        epst = pool.tile([1, 1], F32, tag="epst")
        nc.vector.memset(epst[:], EPS)
        std = pool.tile([1, TOK], F32, tag="std")
        nc.scalar.activation(std[:], var[:], AF.Sqrt, bias=epst[:])
        rstd = pool.tile([1, TOK], F32, tag="rstd")
        nc.vector.reciprocal(rstd[:], std[:])
        ms = pool.tile([1, TOK], F32, tag="ms")
        nc.vector.tensor_tensor(ms[:], mu[:], rstd[:], op=MULT)
        ps_rs = psum.tile([128, 1024], F32, tag="pc0")
        ps_ms = psum.tile([128, 1024], F32, tag="pc1")
        for (o, w_) in HP:
            nc.tensor.matmul(ps_rs[:, o:o + w_], ones_r[:], rstd[:, o:o + w_],
                             start=True, stop=True)
            nc.tensor.matmul(ps_ms[:, o:o + w_], ones_r[:], ms[:, o:o + w_],
                             start=True, stop=True)
        hl = pool.tile([128, KT * TOK], BF16, tag="tbig")
        hl3 = hl[:].rearrange("p (k t) -> p k t", k=KT)
        for kk in range(KT):
            t1 = pev.tile([128, TOK], F32, tag="s1")
            nc.vector.tensor_tensor(t1[:], x13[:, kk], ps_rs[:], op=MULT)
            nc.vector.tensor_tensor(hl3[:, kk], t1[:], ps_ms[:], op=SUB)

        # ---- FFN1: h1 = relu(W1.T @ hl + b1)  (bf16, split tiles;
        # h1a reuses the dead xsb slot) ----
        h1a = pool.tile([128, (H // 256) * TOK], BF16, tag="xsb")
        h1b = pool.tile([128, (H // 256) * TOK], BF16, tag="h1b")
        h1a3 = h1a[:].rearrange("p (k t) -> p k t", k=H // 256)
        h1b3 = h1b[:].rearrange("p (k t) -> p k t", k=H // 256)

        def h13(m):
            return h1a3[:, m] if m < H // 256 else h1b3[:, m - H // 256]

        for m in range(H // 128):
            wm = pw.tile([128, KT * 128], BF16, tag="wm")
            nc.sync.dma_start(
                wm[:].rearrange("p (k e) -> p k e", k=KT),
                w1t[:, m * 128:(m + 1) * 128].rearrange(
                    "(k p) e -> p k e", p=128))
            wm3 = wm[:].rearrange("p (k e) -> p k e", k=KT)
            ps = psb.tile([128, 1024], F32, tag="pmm")
            for (o, w_) in HP:
                for kk in range(KT):
                    nc.tensor.matmul(ps[:, o:o + w_], wm3[:, kk],
                                     hl3[:, kk, o:o + w_],
                                     start=(kk == 0), stop=(kk == KT - 1))
            nc.scalar.activation(h13(m), ps[:], AF.Relu,
                                 bias=b1_sb[:, m:m + 1])

        # ---- FFN2 + residual + b2 ----
        HK = H // 128
        for m in range(KT):
            ps = psb.tile([128, 1024], F32, tag="pmm")
            QK = HK // 4
            for q in range(4):
                wm = pw.tile([128, QK * 128], BF16, tag="wm2")
                nc.sync.dma_start(
                    wm[:].rearrange("p (k e) -> p k e", k=QK),
                    w2t[q * (H // 4):(q + 1) * (H // 4),
                        m * 128:(m + 1) * 128].rearrange(
                        "(k p) e -> p k e", p=128))
                wm3 = wm[:].rearrange("p (k e) -> p k e", k=QK)
                for (o, w_) in HP:
                    for kk in range(QK):
                        gk = q * QK + kk
                        nc.tensor.matmul(ps[:, o:o + w_], wm3[:, kk],
                                         h13(gk)[:, o:o + w_],
                                         start=(gk == 0), stop=(gk == HK - 1))
            t2 = pev.tile([128, TOK], F32, tag="s1")
            nc.scalar.activation(t2[:], ps[:], AF.Identity,
                                 bias=b2_sb[:, m:m + 1])
            oo = pev.tile([128, TOK], F32, tag="s2")
            nc.vector.tensor_tensor(oo[:], x13[:, m], t2[:], op=ADD)
            nc.scalar.dma_start(osd[m * 128:(m + 1) * 128, :], oo[:])

    _fix_sync_waits(nc)
    return nc


# ---------------------------------------------------------------------------
def _get_programs():
    if "progs" not in _cache:
        _cache["progs"] = (build_launch_a(), build_launch_b(), build_launch_c())
    return _cache["progs"]


def _launches():
    """(name, program, in_maps) for each launch of the last kernel() call."""
    progs = _get_programs()
    return [("A", progs[0], _cache["maps_a"]),
            ("B", progs[1], _cache["maps_b"]),
            ("C", progs[2], _cache["maps_c"])]


def kernel(**inputs) -> np.ndarray:
    x = np.ascontiguousarray(np.asarray(inputs["x"], np.float32))
    in_proj_w = np.asarray(inputs["in_proj_w"], np.float32)
    conv_w = np.asarray(inputs["conv_w"], np.float32)
    conv_b = np.asarray(inputs["conv_b"], np.float32)
    x_proj_w = np.asarray(inputs["x_proj_w"], np.float32)
    dt_proj_w = np.asarray(inputs["dt_proj_w"], np.float32)
    dt_proj_b = np.asarray(inputs["dt_proj_b"], np.float32)
    A_log = np.asarray(inputs["A_log"], np.float32)
    D_param = np.asarray(inputs["D_param"], np.float32)
    out_proj_w = np.asarray(inputs["out_proj_w"], np.float32)
    ln1_g = np.asarray(inputs["ln1_g"], np.float32)
    ln1_b = np.asarray(inputs["ln1_b"], np.float32)
    ln2_g = np.asarray(inputs["ln2_g"], np.float32)
    ln2_b = np.asarray(inputs["ln2_b"], np.float32)
    w1 = np.asarray(inputs["w1"], np.float32)
    b1 = np.asarray(inputs["b1"], np.float32)
    w2 = np.asarray(inputs["w2"], np.float32)
    b2v = np.asarray(inputs["b2"], np.float32)

    nca, ncb, ncc = _get_programs()

    # ---- host prep ----
    xT = np.ascontiguousarray(x.reshape(BT, D).T)              # [D, BT]
    WxzT = (in_proj_w * ln1_g[None, :]).T.astype(ml_dtypes.bfloat16)
    bias_xz = (in_proj_w @ ln1_b).reshape(2 * E, 1)
    wxpT = x_proj_w.T.astype(ml_dtypes.bfloat16)
    wdtT = dt_proj_w.T.astype(ml_dtypes.bfloat16)
    Aneg = -np.exp(A_log)
    wopT = out_proj_w.T.astype(ml_dtypes.bfloat16)
    W1T = (w1 * ln2_g[None, :]).T.astype(ml_dtypes.bfloat16)
    b1fv = (b1 + w1 @ ln2_b).reshape(H, 1)
    W2Tb = np.ascontiguousarray(w2.T).astype(ml_dtypes.bfloat16)

    # ---- launch A ----
    in_maps_a = []
    for c in range(NC):
        lo, hi = c * TOK, (c + 1) * TOK
        xs_c = np.zeros((D, TOKH), np.float32)
        batch0 = (lo // T) * T
        hlo = max(lo - HALO, batch0)
        nh = lo - hlo
        if nh:
            xs_c[:, HALO - nh:HALO] = xT[:, hlo:lo]
        xs_c[:, HALO:] = xT[:, lo:hi]
        in_maps_a.append(dict(
            xs=xs_c, wxz=WxzT, wxp=wxpT, wdt=wdtT, cw=conv_w,
            cbias=conv_b.reshape(E, 1), dtb=dt_proj_b.reshape(E, 1),
            bxz=bias_xz))
    _cache['maps_a'] = in_maps_a
    ra = run_bass_kernel_spmd(nca, in_maps_a, core_ids=list(range(NC)))

    delta_g = np.concatenate([r["delta_s"] for r in ra.results], axis=1)
    xc_g = np.concatenate([r["xc_s"] for r in ra.results], axis=1)
    zs_g = np.concatenate([r["zs_s"] for r in ra.results], axis=1)
    Bg = np.concatenate([r["bc_s"] for r in ra.results], axis=1)  # bf16
    Cg = np.concatenate([r["cc_s"] for r in ra.results], axis=1)

    # n-major per chunk: [NCH, N, TC]
    bfl = np.ascontiguousarray(
        Bg.reshape(N, NCH, TC).transpose(1, 0, 2)).reshape(1, -1)
    cfl = np.ascontiguousarray(
        Cg.reshape(N, NCH, TC).transpose(1, 0, 2)).reshape(1, -1)

    in_maps_b = []
    for c in range(NC):
        elo, ehi = c * ES, (c + 1) * ES
        in_maps_b.append(dict(
            dl=np.ascontiguousarray(delta_g[elo:ehi]),
            xc=np.ascontiguousarray(xc_g[elo:ehi]),
            zs=np.ascontiguousarray(zs_g[elo:ehi]),
            bfl=bfl, cfl=cfl,
            aneg=np.ascontiguousarray(Aneg[elo:ehi]),
            dpr=np.ascontiguousarray(D_param[elo:ehi].reshape(ES, 1))))
    _cache['maps_b'] = in_maps_b
    rb = run_bass_kernel_spmd(ncb, in_maps_b, core_ids=list(range(NC)))
    y_g = np.concatenate([r["ys"] for r in rb.results], axis=0)  # [E, BT]

    in_maps_c = []
    for c in range(NC):
        lo, hi = c * TOK, (c + 1) * TOK
        in_maps_c.append(dict(
            ysd=np.ascontiguousarray(y_g[:, lo:hi]),
            xsd=np.ascontiguousarray(xT[:, lo:hi]),
            wop=wopT, w1t=W1T, w2t=W2Tb, b1f=b1fv,
            b2=b2v.reshape(D, 1)))
    _cache['maps_c'] = in_maps_c
    rc = run_bass_kernel_spmd(ncc, in_maps_c, core_ids=list(range(NC)))
    out_cm = np.concatenate([r["osd"] for r in rc.results], axis=1)  # [D, BT]

    return np.ascontiguousarray(out_cm.T).reshape(B, T, D).astype(np.float32)

